# revision 10
# baseline (speedup 1.0000x reference)
"""Trainium2 Bass kernel for the CNN-MAD per-class DTW transport cost.

Math (reference):
  mat_cost[n, j] = C1[n] + C2[c_n, j] - 2*C3[n, j],  c_n = classes[n]
    C1[n]    = sum_t rowsum[c_n, t] * ||X[n,t,:]||^2
    C2[c, j] = sum_p colsum[c, p] * ||Y[j,p,:]||^2
    C3[n, j] = sum_{p,d} (sum_t pi[c_n,t,p] X[n,t,d]) * Y[j,p,d]

Sharding: one class per core (C == n_cores == 8). Host groups samples by
class (pure gather / re-layout, no arithmetic), each core computes the
[CAP, NY] block of rows for its class against the full Y, and the host
scatters rows back into the [N, NY] output.

Device per core (class k), all f32:
  - pi    [T, TP]          : class-k DTW matrix
  - xt2   [T, D*CAP]       : X.T re-layout, xt2[t, d*CAP+n] = Xg[n, t, d]
  - yt    [D*TP, NY]       : Y.T re-layout, yt[d*TP+p, j]  = Y[j, p, d]
  rowsum via DVE reduce, colsum via matmul with ones, C1 via matmul with
  rowsum over squared xt2, C2 via matmul with colsum over squared yt
  chunks, XW = pi.T @ X per d via matmul, and the final [CAP, NY] result
  as one PSUM accumulation: sum_k (-2*XW)^T yt + [C1;1]^T [1;C2].
"""

import sys

sys.path.insert(0, "/opt/trn_rl_repo")

import numpy as np

N, NY, T, TP, D, C = 1024, 1024, 256, 256, 8, 8
NCORES = 8

_cache = {}


def _build(cap):
    import concourse.bacc as bacc
    import concourse.mybir as mybir
    import concourse.tile as tile

    dt = mybir.dt.float32
    nc = bacc.Bacc("TRN2", target_bir_lowering=False, debug=False, num_devices=NCORES)

    pi_d = nc.dram_tensor("pi", [T, TP], dt, kind="ExternalInput")
    xt2_d = nc.dram_tensor("xt2", [T, D * cap], dt, kind="ExternalInput")
    yt_d = nc.dram_tensor("yt", [D * TP, NY], dt, kind="ExternalInput")
    out_d = nc.dram_tensor("out", [cap, NY], dt, kind="ExternalOutput")

    KC = D * TP // 128  # 16 yt chunks of 128 contraction rows
    XF = D * cap        # xt2 free size
    # n-partition tiles of the CAP samples
    ntiles = [(0, 128), (128, cap - 128)] if cap > 128 else [(0, cap)]

    with tile.TileContext(nc) as tc:
        with (
            tc.tile_pool(name="const", bufs=1) as pconst,
            tc.tile_pool(name="xin", bufs=1) as px,
            tc.tile_pool(name="ytp", bufs=1) as pyt,
            tc.tile_pool(name="ytsq", bufs=3) as pytsq,
            tc.tile_pool(name="xwt", bufs=1) as pxwt,
            tc.tile_pool(name="osb", bufs=3) as posb,
            tc.tile_pool(name="ps_a", bufs=2, space="PSUM") as ps_a,
            tc.tile_pool(name="ps_c2", bufs=1, space="PSUM") as ps_c2,
            tc.tile_pool(name="ps_c3", bufs=2, space="PSUM") as ps_c3,
        ):
            # ---- load pi, compute rowsum (free-dim reduce) ----
            pi_sb = []
            rowsum = []
            for tch in range(2):
                p = pconst.tile([128, TP], dt, tag=f"pi{tch}")
                nc.gpsimd.dma_start(p[:], pi_d[tch * 128 : (tch + 1) * 128, :])
                pi_sb.append(p)
                r = pconst.tile([128, 1], dt, tag=f"rowsum{tch}")
                nc.vector.reduce_sum(r[:], p[:], axis=mybir.AxisListType.X)
                rowsum.append(r)

            ones = pconst.tile([128, 1], dt, tag="ones")
            nc.gpsimd.memset(ones[:], 1.0)

            # ---- colsum via matmul with ones: [128p, 1] per p-half ----
            colsum = []
            for pc in range(2):
                ps = ps_a.tile([128, 1], dt, tag="psa")
                for tch in range(2):
                    nc.tensor.matmul(
                        ps[:],
                        pi_sb[tch][:, pc * 128 : (pc + 1) * 128],
                        ones[:],
                        start=(tch == 0),
                        stop=(tch == 1),
                    )
                cs = pconst.tile([128, 1], dt, tag=f"colsum{pc}")
                nc.vector.tensor_copy(cs[:], ps[:])
                colsum.append(cs)

            # ---- load xt2, square it ----
            xt2 = []
            xt2sq = []
            for tch in range(2):
                xt = px.tile([128, XF], dt, tag=f"xt2_{tch}")
                nc.gpsimd.dma_start(xt[:], xt2_d[tch * 128 : (tch + 1) * 128, :])
                xt2.append(xt)
                xsq = px.tile([128, XF], dt, tag=f"xt2sq_{tch}")
                nc.scalar.square(xsq[:], xt[:])
                xt2sq.append(xsq)

            # ---- C1 row: sum_{t,d} rowsum[t] * xt2sq[t, (d,n)] -> [1, cap] ----
            c1row = pconst.tile([1, cap], dt, tag="c1row")
            for d in range(D):
                ps = ps_a.tile([1, cap], dt, tag="psa")
                for tch in range(2):
                    nc.tensor.matmul(
                        ps[0:1, :],
                        rowsum[tch][:],
                        xt2sq[tch][:, d * cap : (d + 1) * cap],
                        start=(tch == 0),
                        stop=(tch == 1),
                    )
                if d == 0:
                    nc.vector.tensor_copy(c1row[0:1, :], ps[0:1, :])
                else:
                    nc.vector.tensor_add(c1row[0:1, :], c1row[0:1, :], ps[0:1, :])
            ones_cap = pconst.tile([1, cap], dt, tag="ones_cap")
            nc.gpsimd.memset(ones_cap[:], 1.0)

            # ---- XW: per (d, p-half): [128p, cap] = pi_half.T @ xt2_d ----
            # xwt chunk kc = d*2+pc holds (-2*XW).T rows [kc*128, (kc+1)*128)
            xwt = pxwt.tile([128, KC * cap], dt, tag="xwt")
            for d in range(D):
                for pc in range(2):
                    kc = d * 2 + pc
                    ps = ps_a.tile([128, cap], dt, tag="psa")
                    for tch in range(2):
                        nc.tensor.matmul(
                            ps[:],
                            pi_sb[tch][:, pc * 128 : (pc + 1) * 128],
                            xt2[tch][:, d * cap : (d + 1) * cap],
                            start=(tch == 0),
                            stop=(tch == 1),
                        )
                    nc.vector.tensor_scalar_mul(
                        xwt[:, kc * cap : (kc + 1) * cap], ps[:], -2.0
                    )

            # ---- load yt chunks; C2 accumulation over squared chunks ----
            yt = pyt.tile([128, KC * NY], dt, tag="yt")
            for kc in range(KC):
                nc.gpsimd.dma_start(
                    yt[:, kc * NY : (kc + 1) * NY],
                    yt_d[kc * 128 : (kc + 1) * 128, :],
                )

            c2ps = ps_c2.tile([1, NY], dt, tag="c2ps")
            for kc in range(KC):
                ysq = pytsq.tile([128, NY], dt, tag="ysq")
                nc.scalar.square(ysq[:], yt[:, kc * NY : (kc + 1) * NY])
                for jb in range(2):
                    nc.tensor.matmul(
                        c2ps[0:1, jb * 512 : (jb + 1) * 512],
                        colsum[kc % 2][:],
                        ysq[:, jb * 512 : (jb + 1) * 512],
                        start=(kc == 0),
                        stop=(kc == KC - 1),
                    )

            c2row = pconst.tile([1, NY], dt, tag="c2row")
            nc.vector.tensor_copy(c2row[0:1, :], c2ps[0:1, :])
            ones_ny = pconst.tile([1, NY], dt, tag="ones_ny")
            nc.gpsimd.memset(ones_ny[:], 1.0)

            # ---- C3 + aug: out[n, j] = sum_kc xwt_kc.T @ yt_kc + aug ----
            for n0, nn_ in ntiles:
                for jb in range(2):
                    ps = ps_c3.tile([128, 512], dt, tag="c3ps")
                    for kc in range(KC):
                        nc.tensor.matmul(
                            ps[:nn_, :],
                            xwt[:, kc * cap + n0 : kc * cap + n0 + nn_],
                            yt[:, kc * NY + jb * 512 : kc * NY + (jb + 1) * 512],
                            start=(kc == 0),
                            stop=False,
                        )
                    nc.tensor.matmul(
                        ps[:nn_, :],
                        c1row[:, n0 : n0 + nn_],
                        ones_ny[:, jb * 512 : (jb + 1) * 512],
                        start=False,
                        stop=False,
                    )
                    nc.tensor.matmul(
                        ps[:nn_, :],
                        ones_cap[:, n0 : n0 + nn_],
                        c2row[:, jb * 512 : (jb + 1) * 512],
                        start=False,
                        stop=True,
                    )
                    osb = posb.tile([128, 512], dt, tag="osb")
                    nc.vector.tensor_copy(osb[:nn_, :], ps[:nn_, :])
                    nc.gpsimd.dma_start(
                        out_d[n0 : n0 + nn_, jb * 512 : (jb + 1) * 512], osb[:nn_, :]
                    )

    nc.compile()
    return nc


def kernel(X, Y, pi_dtw, classes):
    from concourse.bass_utils import run_bass_kernel_spmd

    X = np.ascontiguousarray(np.asarray(X, dtype=np.float32))
    Y = np.ascontiguousarray(np.asarray(Y, dtype=np.float32))
    pi_dtw = np.ascontiguousarray(np.asarray(pi_dtw, dtype=np.float32))
    classes = np.asarray(classes).astype(np.int64)

    counts = np.bincount(classes, minlength=C)
    cap = max(160, int(-(-counts.max() // 32) * 32))

    key = cap
    if key not in _cache:
        _cache[key] = _build(cap)
    nc = _cache[key]

    # host-side re-layouts (data movement only, no arithmetic)
    yt = np.ascontiguousarray(Y.transpose(2, 1, 0).reshape(D * TP, NY))
    idx = [np.nonzero(classes == c)[0] for c in range(C)]
    in_maps = []
    for c in range(C):
        xg = np.zeros((cap, T, D), dtype=np.float32)
        xg[: counts[c]] = X[idx[c]]
        xt2 = np.ascontiguousarray(xg.transpose(1, 2, 0).reshape(T, D * cap))
        in_maps.append(
            {"pi": np.ascontiguousarray(pi_dtw[c]), "xt2": xt2, "yt": yt}
        )

    res = run_bass_kernel_spmd(nc, in_maps, core_ids=list(range(NCORES)))

    out = np.empty((N, NY), dtype=np.float32)
    for c in range(C):
        out[idx[c]] = res.results[c]["out"][: counts[c]]
    return out


# revision 13
# speedup vs baseline: 1.5101x; 1.5101x over previous
"""Trainium2 Bass kernel for the CNN-MAD per-class DTW transport cost.

Math (reference):
  mat_cost[n, j] = C1[n] + C2[c_n, j] - 2*C3[n, j],  c_n = classes[n]
    C1[n]    = sum_t rowsum[c_n, t] * ||X[n,t,:]||^2
    C2[c, j] = sum_p colsum[c, p] * ||Y[j,p,:]||^2
    C3[n, j] = sum_{p,d} (sum_t pi[c_n,t,p] X[n,t,d]) * Y[j,p,d]

Sharding: one class per core (C == n_cores == 8). Host groups samples by
class (pure gather / re-layout, no arithmetic), each core computes the
[NY, CAP] transposed block for its class against the full Y, and the host
scatters rows back into the [N, NY] output.

Device per core (class k), all f32:
  - pi    [T, TP]      : class-k DTW matrix
  - xt2   [T, D*CAP]   : X.T re-layout, xt2[t, d*CAP+n] = Xg[n, t, d]
  - yt    [D*TP, NY]   : Y.T re-layout, yt[d*TP+p, j]  = Y[j, p, d]
  rowsum via DVE reduce; colsum via matmul with ones; C1 via matmul of
  rowsum over squared xt2; C2 via ACT squares scaled by sqrt(colsum) and
  a DVE partition-chunk sum followed by a ones-contraction matmul;
  XW = pi.T @ X per d; final transposed result outT[j, n] accumulated in
  PSUM as sum_kc yt_kc.T @ (-2*XW)_kc + [C2;1].T [1;C1].
"""

import sys

sys.path.insert(0, "/opt/trn_rl_repo")

import numpy as np

N, NY, T, TP, D, C = 1024, 1024, 256, 256, 8, 8
NCORES = 8

_cache = {}


def _build(cap):
    import concourse.bacc as bacc
    import concourse.mybir as mybir
    import concourse.tile as tile

    dt = mybir.dt.float32
    nc = bacc.Bacc("TRN2", target_bir_lowering=False, debug=False, num_devices=NCORES)

    pi_d = nc.dram_tensor("pi", [T, TP], dt, kind="ExternalInput")
    xt2_d = nc.dram_tensor("xt2", [T, D * cap], dt, kind="ExternalInput")
    yt_d = nc.dram_tensor("yt", [D * TP, NY], dt, kind="ExternalInput")
    out_d = nc.dram_tensor("outT", [NY, cap], dt, kind="ExternalOutput")

    KC = D * TP // 128  # 16 yt chunks of 128 contraction rows
    XF = D * cap        # xt2 free size
    JT = NY // 128      # 8 output partition tiles (transposed layout)

    with tile.TileContext(nc) as tc:
        with (
            tc.tile_pool(name="const", bufs=1) as pconst,
            tc.tile_pool(name="xin", bufs=1) as px,
            tc.tile_pool(name="ytp", bufs=1) as pyt,
            tc.tile_pool(name="ysqw", bufs=3) as pysq,
            tc.tile_pool(name="xwt", bufs=1) as pxwt,
            tc.tile_pool(name="osb", bufs=3) as posb,
            tc.tile_pool(name="ps_a", bufs=1, space="PSUM") as ps_a,
            tc.tile_pool(name="ps_xw", bufs=1, space="PSUM") as ps_xw,
            tc.tile_pool(name="ps_c2", bufs=1, space="PSUM") as ps_c2,
            tc.tile_pool(name="ps_o", bufs=2, space="PSUM") as ps_o,
        ):
            # ---- load pi, rowsum (free-dim reduce) ----
            pi_sb = []
            rowsum = []
            for tch in range(2):
                p = pconst.tile([128, TP], dt, tag=f"pi{tch}")
                nc.gpsimd.dma_start(p[:], pi_d[tch * 128 : (tch + 1) * 128, :])
                pi_sb.append(p)
                r = pconst.tile([128, 1], dt, tag=f"rowsum{tch}")
                nc.vector.reduce_sum(r[:], p[:], axis=mybir.AxisListType.X)
                rowsum.append(r)

            ones = pconst.tile([128, 1], dt, tag="ones")
            nc.gpsimd.memset(ones[:], 1.0)

            # ---- colsum + sqrt(colsum) per p-half ----
            sqrtcol = []
            for pc in range(2):
                ps = ps_a.tile([128, 1], dt, tag="psa")
                for tch in range(2):
                    nc.tensor.matmul(
                        ps[:],
                        pi_sb[tch][:, pc * 128 : (pc + 1) * 128],
                        ones[:],
                        start=(tch == 0),
                        stop=(tch == 1),
                    )
                sc = pconst.tile([128, 1], dt, tag=f"sqrtcol{pc}")
                nc.scalar.sqrt(sc[:], ps[:])
                sqrtcol.append(sc)

            # ---- load xt2, square ----
            xt2 = []
            xt2sq = []
            for tch in range(2):
                xt = px.tile([128, XF], dt, tag=f"xt2_{tch}")
                nc.gpsimd.dma_start(xt[:], xt2_d[tch * 128 : (tch + 1) * 128, :])
                xt2.append(xt)
                xsq = px.tile([128, XF], dt, tag=f"xt2sq_{tch}")
                nc.scalar.square(xsq[:], xt[:])
                xt2sq.append(xsq)

            # ---- C1 row: sum_{t,d} rowsum[t] * xt2sq[t, (d,n)] -> [1, cap] ----
            c1row = pconst.tile([1, cap], dt, tag="c1row")
            for d in range(D):
                ps = ps_a.tile([1, cap], dt, tag="psa")
                for tch in range(2):
                    nc.tensor.matmul(
                        ps[0:1, :],
                        rowsum[tch][:],
                        xt2sq[tch][:, d * cap : (d + 1) * cap],
                        start=(tch == 0),
                        stop=(tch == 1),
                    )
                if d == 0:
                    nc.vector.tensor_copy(c1row[0:1, :], ps[0:1, :])
                else:
                    nc.vector.tensor_add(c1row[0:1, :], c1row[0:1, :], ps[0:1, :])

            # aug rhs [2, cap]: row0 = ones, row1 = C1row (via SBUF->SBUF DMA
            # because compute engines cannot write at partition base 1)
            aug_r = pconst.tile([2, cap], dt, tag="aug_r")
            nc.gpsimd.memset(aug_r[:], 1.0)
            nc.gpsimd.dma_start(aug_r[1:2, :], c1row[0:1, :])

            # ---- XW: per p-half, out [128p, (d,n)] = pi_half.T @ xt2 ----
            # xwt holds (-2*XW).T chunk kc = d*2+pc at [:, kc*cap:(kc+1)*cap]
            xwt = pxwt.tile([128, KC * cap], dt, tag="xwt")
            for pc in range(2):
                ps = ps_xw.tile([128, XF], dt, tag="psxw")
                seg = 0
                while seg < XF:
                    w = min(512, XF - seg)
                    for tch in range(2):
                        nc.tensor.matmul(
                            ps[:, seg : seg + w],
                            pi_sb[tch][:, pc * 128 : (pc + 1) * 128],
                            xt2[tch][:, seg : seg + w],
                            start=(tch == 0),
                            stop=(tch == 1),
                        )
                    seg += w
                # scatter (d, n) segments into chunk layout with -2 scale
                for d in range(D):
                    kc = d * 2 + pc
                    nc.vector.tensor_scalar_mul(
                        xwt[:, kc * cap : (kc + 1) * cap],
                        ps[:, d * cap : (d + 1) * cap],
                        -2.0,
                    )

            # ---- load yt chunks; squared+scaled partial sums for C2 ----
            yt = pyt.tile([128, KC * NY], dt, tag="yt")
            for kc in range(KC):
                nc.gpsimd.dma_start(
                    yt[:, kc * NY : (kc + 1) * NY],
                    yt_d[kc * 128 : (kc + 1) * 128, :],
                )

            # s[l, j] = sum_kc colsum[kc%2][l] * yt[kc][l, j]^2 via ACT Square
            # with per-partition scale sqrt(colsum), then a DVE add chain.
            ssum = pconst.tile([128, NY], dt, tag="ssum")
            for kc in range(KC):
                if kc == 0:
                    nc.scalar.activation(
                        ssum[:],
                        yt[:, 0:NY],
                        mybir.ActivationFunctionType.Square,
                        scale=sqrtcol[0][:],
                    )
                else:
                    ysq = pysq.tile([128, NY], dt, tag="ysq")
                    nc.scalar.activation(
                        ysq[:],
                        yt[:, kc * NY : (kc + 1) * NY],
                        mybir.ActivationFunctionType.Square,
                        scale=sqrtcol[kc % 2][:],
                    )
                    nc.vector.tensor_add(ssum[:], ssum[:], ysq[:])

            # C2 row [1, NY]: ones-contraction of ssum
            c2ps = ps_c2.tile([1, NY], dt, tag="c2ps")
            for jb in range(2):
                nc.tensor.matmul(
                    c2ps[0:1, jb * 512 : (jb + 1) * 512],
                    ones[:],
                    ssum[:, jb * 512 : (jb + 1) * 512],
                    start=True,
                    stop=True,
                )
            # aug lhsT [2, NY]: row0 = C2row, row1 = ones
            aug_l = pconst.tile([2, NY], dt, tag="aug_l")
            nc.gpsimd.memset(aug_l[:], 1.0)
            nc.vector.tensor_copy(aug_l[0:1, :], c2ps[0:1, :])

            # ---- C3 transposed: outT[j, n] ----
            for jt in range(JT):
                ps = ps_o.tile([128, cap], dt, tag="c3ps")
                for kc in range(KC):
                    nc.tensor.matmul(
                        ps[:],
                        yt[:, kc * NY + jt * 128 : kc * NY + (jt + 1) * 128],
                        xwt[:, kc * cap : (kc + 1) * cap],
                        start=(kc == 0),
                        stop=False,
                    )
                nc.tensor.matmul(
                    ps[:],
                    aug_l[:, jt * 128 : (jt + 1) * 128],
                    aug_r[:],
                    start=False,
                    stop=True,
                )
                osb = posb.tile([128, cap], dt, tag="osb")
                nc.vector.tensor_copy(osb[:], ps[:])
                nc.gpsimd.dma_start(out_d[jt * 128 : (jt + 1) * 128, :], osb[:])

    nc.compile()
    return nc


def kernel(X, Y, pi_dtw, classes):
    from concourse.bass_utils import run_bass_kernel_spmd

    X = np.ascontiguousarray(np.asarray(X, dtype=np.float32))
    Y = np.ascontiguousarray(np.asarray(Y, dtype=np.float32))
    pi_dtw = np.ascontiguousarray(np.asarray(pi_dtw, dtype=np.float32))
    classes = np.asarray(classes).astype(np.int64)

    counts = np.bincount(classes, minlength=C)
    cap = max(160, int(-(-counts.max() // 32) * 32))

    if cap not in _cache:
        _cache[cap] = _build(cap)
    nc = _cache[cap]

    # host-side re-layouts (data movement only, no arithmetic)
    yt = np.ascontiguousarray(Y.transpose(2, 1, 0).reshape(D * TP, NY))
    idx = [np.nonzero(classes == c)[0] for c in range(C)]
    in_maps = []
    for c in range(C):
        xg = np.zeros((cap, T, D), dtype=np.float32)
        xg[: counts[c]] = X[idx[c]]
        xt2 = np.ascontiguousarray(xg.transpose(1, 2, 0).reshape(T, D * cap))
        in_maps.append(
            {"pi": np.ascontiguousarray(pi_dtw[c]), "xt2": xt2, "yt": yt}
        )

    res = run_bass_kernel_spmd(nc, in_maps, core_ids=list(range(NCORES)))

    out = np.empty((N, NY), dtype=np.float32)
    for c in range(C):
        out[idx[c]] = res.results[c]["outT"].T[: counts[c]]
    return out
